# revision 23
# baseline (speedup 1.0000x reference)
"""Trainium2 Bass kernel for the forward-attention LSA step (nn_LSA_43404939494068).

Contract: kernel(**inputs) takes the FULL inputs from setup_inputs() and
returns the FULL output [64, 1, 1024] float32. Internally shards batch
across 8 NeuronCores (8 batches each), runs one Bass/Tile program SPMD.

Math notes (vs reference):
  u[b,t]   = sum_a v[a] * tanh(pq[b,a] + enc[b,t,a] + ploc[b,t,a])
  ploc     = conv1d([cumulative; attention]) @ L_w.T + L_b; the conv and the
             L-projection fold into ONE matmul: ploc[t,:] = ls[:,t].T @ M,
             M[(c,k),a] = sum_f conv_w[f,c,k] * L_w[a,f] (host-precomputed
             weight algebra), ls = 62 shifted copies of the two loc rows.
  pq+L_b+W_b is computed on host (tiny: B x A) and folded into enc, which is
  shipped bf16 (halves HBM traffic; final rel-err stays ~1e-3 << 2e-2).
  The reference's division of s=sigmoid(u) by sum(s) cancels exactly in the
  final alpha normalization, so it is skipped.

Engine placement, tuned against wall-differenced hardware timings (the
CoreSim cost model misses the ~400ns weight-load+accumulate cost of a
128-row matmul, which made the enc identity-matmul the real bottleneck):
per [128t x 2x512a] PSUM pair, PE runs the bf16 folded conv+proj matmul;
enc joins either via a bf16 identity-matmul into the same PSUM bank (2 of
4 pairs) or via a DVE tensor_add (other 2 pairs) - the 50/50 split
balances measured PE vs DVE time. One ACT tanh covers each 2-bank pair
(amortizes PSUM access latency); the x*v dot is a DVE
scalar_tensor_tensor with accum writing one u column (Pool cannot run
vector ops on NCv3 - walrus ISA check). The tail (sigmoid, premultiplied
mask*(alpha-band+eps), normalize) runs in a [t',(k,b)] 64-column layout,
then one PE transpose and a single output DMA. Input DMAs are spread
across queues: enc owns SP, first-use constants go on the ACT queue,
the rest on the gpsimd SWDGE queue; the ACT tanh table is preloaded with
a dummy activation at t=0.
"""

import sys

import numpy as np

if "/opt/trn_rl_repo" not in sys.path:
    sys.path.insert(0, "/opt/trn_rl_repo")

import concourse.bass as bass
import concourse.tile as tile
from concourse import mybir
from concourse.bass_utils import run_bass_kernel_spmd

B, T, A = 64, 1024, 512
F, KW = 32, 31
PAD = (KW - 1) // 2
NCORES = 8
LB = B // NCORES          # 8 local batches per core
NK = T // 128             # 8 t-tiles of 128
KC = 62                   # conv contraction = 2 channels * 31 taps
F32 = mybir.dt.float32
F32R = mybir.dt.float32r
BF16 = mybir.dt.bfloat16

# const blob layout (fp32, [128, 640]): tri | cor | ones | eye32 | mask | alpha
C_TRI, C_COR, C_ONES, C_EYE = 0, 128, 256, 384
C_MASK, C_ALPHA = 512, 576
C_W = 640

_MAX_WAITS = 1


def _split_sync_waits(nc):
    """walrus in this toolchain accepts at most one sync-wait per
    instruction; hoist excess waits onto NoOps inserted just before."""
    for fn in nc.m.functions:
        for blk in fn.blocks:
            new_list = []
            for inst in blk.instructions:
                si = inst.sync_info
                if si is not None and si.on_wait and len(si.on_wait) > _MAX_WAITS:
                    waits = list(si.on_wait)
                    extra, keep = waits[:-_MAX_WAITS], waits[-_MAX_WAITS:]
                    for i in range(0, len(extra), _MAX_WAITS):
                        nop = mybir.InstNoOp(
                            name=nc.get_next_instruction_name(),
                            sync_info=mybir.SyncInfo(
                                on_wait=extra[i:i + _MAX_WAITS], on_update=[]
                            ),
                            bass_nofuse=True,
                            engine=inst.engine,
                        )
                        nc.register_instruction(nop)
                        new_list.append(nop)
                    inst.sync_info = mybir.SyncInfo(
                        on_wait=keep, on_update=list(si.on_update)
                    )
                new_list.append(inst)
            blk.instructions[:] = new_list


def build_program(repeats: int = 1, hw_loop: bool = False,
                  stage: str = "full", unroll: int = 1) -> bass.Bass:
    nc = bass.Bass()

    enc_d = nc.declare_dram_parameter("enc", [LB, T, A], BF16, isOutput=False)
    ls_d = nc.declare_dram_parameter("ls", [KC, LB, T], BF16, isOutput=False)
    mcomb_d = nc.declare_dram_parameter("mcomb", [KC, A], BF16, isOutput=False)
    vw_d = nc.declare_dram_parameter("vw", [A], BF16, isOutput=False)
    eyeb_d = nc.declare_dram_parameter("eyeb", [128, 128], BF16, isOutput=False)
    const_d = nc.declare_dram_parameter("constblob", [128, C_W], F32, isOutput=False)
    out_d = nc.declare_dram_parameter("out", [LB * NK, 128], F32, isOutput=True)

    TANH = mybir.ActivationFunctionType.Tanh
    SIG = mybir.ActivationFunctionType.Sigmoid
    IDENT = mybir.ActivationFunctionType.Identity
    MULT = mybir.AluOpType.mult
    ADD = mybir.AluOpType.add

    with tile.TileContext(nc) as tc:
        with (
            tc.tile_pool(name="const", bufs=1) as cpool,
            tc.tile_pool(name="encp", bufs=4) as encp,
            tc.tile_pool(name="xp", bufs=6) as xp,
            tc.tile_pool(name="tailp", bufs=3) as tailp,
            tc.tile_pool(name="zps", bufs=3, space="PSUM") as zps,
            tc.tile_pool(name="sps", bufs=2, space="PSUM") as sps,
        ):
            # ---- constants into SBUF (gpsimd/SWDGE queue; enc owns the SP
            # queue). Order = first-use order: the b0 matmuls need mcomb+eyeb
            # +ls[0] immediately; the const blob is tail-only.
            mcomb_sb = cpool.tile([KC, A], BF16, tag="mcomb")
            nc.scalar.dma_start(out=mcomb_sb[:], in_=mcomb_d[:])
            eyeb_sb = cpool.tile([128, 128], BF16, tag="eyeb")
            nc.scalar.dma_start(out=eyeb_sb[:], in_=eyeb_d[:])

            ls_sb = cpool.tile([KC, LB, T], BF16, tag="ls")
            nc.scalar.dma_start(out=ls_sb[:, 0, :], in_=ls_d[:, 0, :])

            # v broadcast to all 128 partitions (partition-step-0 DMA)
            v_sb = cpool.tile([128, A], BF16, tag="vbc")
            va = vw_d[:]
            v_bcast = bass.AP(tensor=va.tensor, offset=va.offset,
                              ap=[[0, 128]] + [list(p) for p in va.ap])
            nc.scalar.dma_start(out=v_sb[:], in_=v_bcast)

            for b in range(1, LB):
                nc.gpsimd.dma_start(out=ls_sb[:, b, :], in_=ls_d[:, b, :])
            const_sb = cpool.tile([128, C_W], F32, tag="const")
            nc.gpsimd.dma_start(out=const_sb[:], in_=const_d[:])

            u_sb = cpool.tile([128, LB * NK], F32, tag="u")
            eps_sb = cpool.tile([128, 1], F32, tag="eps")
            nc.vector.memset(eps_sb[:], 1e-7)
            warm_sb = cpool.tile([128, 1], F32, tag="warm")
            nc.scalar.activation(out=warm_sb[:], in_=eps_sb[:], func=TANH)

            tri = const_sb[:, C_TRI:C_TRI + 128]
            cor = const_sb[:, C_COR:C_COR + 128]
            ones = const_sb[:, C_ONES:C_ONES + 128]
            eye32 = const_sb[:, C_EYE:C_EYE + 128]
            mask = const_sb[:, C_MASK:C_MASK + LB * NK]
            alpha = const_sb[:, C_ALPHA:C_ALPHA + LB * NK]

            def body():
                # ---- main loop: z = ploc+(pq+enc) ; x = tanh(z) ; u = x.v ----
                for b in range(LB):
                    enc_sb = encp.tile([128, NK, A], BF16, tag="enc")
                    src_enc = enc_d[b].rearrange("(k p) a -> p k a", p=128)
                    if b == 0:
                        nc.sync.dma_start(out=enc_sb[:, 0:2, :],
                                          in_=src_enc[:, 0:2, :])
                        nc.sync.dma_start(out=enc_sb[:, 2:, :],
                                          in_=src_enc[:, 2:, :])
                    else:
                        nc.sync.dma_start(out=enc_sb[:], in_=src_enc)
                    if stage == "dma":
                        continue
                    for kp in range(NK // 2):
                        # two k-tiles share one 2-bank PSUM tile so a single
                        # tanh covers both (amortizes ACT access latency)
                        # dve_add pairs: enc joins via a DVE add instead of
                        # the PE identity-matmul (balances PE vs DVE load)
                        dve_add = ((stage in ("full", "d1") and kp == 3)
                                   or (stage == "fullh" and kp % 2 == 1)
                                   or (stage == "d3" and kp >= 1))
                        z_ps = zps.tile([128, 2, A], F32, tag="z")
                        # both mm1 first, then both identity-matmuls: the
                        # scheduler elides the second (identical) eyeb
                        # weight load when the two are adjacent
                        for j in range(2):
                            k = 2 * kp + j
                            nc.tensor.matmul(
                                z_ps[:, j, :],
                                ls_sb[:, b, k * 128:(k + 1) * 128],
                                mcomb_sb[:],
                                start=True, stop=(dve_add or stage == "noeye"),
                                skip_group_check=True)
                        if not (dve_add or stage == "noeye"):
                            for j in range(2):
                                k = 2 * kp + j
                                nc.tensor.matmul(z_ps[:, j, :], eyeb_sb[:],
                                                 enc_sb[:, k, :],
                                                 start=False, stop=True,
                                                 skip_group_check=True)
                        if stage == "mm":
                            continue
                        x_sb = xp.tile([128, 2, A], BF16, tag="x")
                        if dve_add:
                            xin_sb = xp.tile([128, 2, A], BF16, tag="xin")
                            nc.vector.tensor_add(
                                xin_sb[:], z_ps[:],
                                enc_sb[:, 2 * kp:2 * kp + 2, :])
                            nc.scalar.activation(out=x_sb[:], in_=xin_sb[:],
                                                 func=TANH)
                        else:
                            nc.scalar.activation(out=x_sb[:], in_=z_ps[:],
                                                 func=TANH)
                        if stage == "act":
                            continue
                        for j in range(2):
                            # in-place x *= v: no separate xv tile, so the
                            # stt carries no buffer-rotation wait (x is dead
                            # after this op; the x-pool WAR already orders
                            # the next tanh against it)
                            k = 2 * kp + j
                            col = k * LB + b
                            nc.vector.scalar_tensor_tensor(
                                out=x_sb[:, j, :], in0=x_sb[:, j, :],
                                scalar=1.0, in1=v_sb[:], op0=MULT, op1=MULT,
                                accum_out=u_sb[:, col:col + 1])

                if stage != "full":
                    dum_sb = tailp.tile([LB * NK, 128], F32, tag="otsb")
                    nc.vector.tensor_copy(out=dum_sb[:],
                                          in_=const_sb[0:LB * NK, 0:128])
                    nc.sync.dma_start(out=out_d[:], in_=dum_sb[:])
                    return
                # ---- tail in [t', (k,b)] layout ----
                # wm = (alpha + shift1(alpha) + shift2(alpha) + eps) * mask
                # depends only on constants: runs under the main loop, off
                # the post-u critical path (band shifts via tri/cor matmuls)
                w_ps = sps.tile([128, LB * NK], F32, tag="tailps")
                nc.tensor.matmul(w_ps[:], tri, alpha, start=True, stop=False)
                nc.tensor.matmul(w_ps[:, LB:], cor, alpha[:, :-LB],
                                 start=False, stop=True, skip_group_check=True)
                wm_sb = tailp.tile([128, LB * NK], F32, tag="wm")
                nc.vector.scalar_tensor_tensor(
                    out=wm_sb[:], in0=w_ps[:], scalar=1e-7, in1=mask,
                    op0=ADD, op1=MULT)

                s_sb = tailp.tile([128, LB * NK], F32, tag="s")
                nc.scalar.activation(out=s_sb[:], in_=u_sb[:], func=SIG)
                na_sb = tailp.tile([128, LB * NK], F32, tag="na")
                nc.vector.tensor_mul(na_sb[:], s_sb[:], wm_sb[:])

                # per-batch normalizer: colsum then sum over the k-groups
                cs_ps = sps.tile([1, LB * NK], F32, tag="tailps")
                nc.tensor.matmul(cs_ps[:], ones[:, 0:1], na_sb[:],
                                 start=True, stop=True)
                z_sb = tailp.tile([1, LB], F32, tag="zsum")
                nc.vector.tensor_reduce(
                    out=z_sb[:], in_=cs_ps.rearrange("p (k b) -> p b k", b=LB),
                    axis=mybir.AxisListType.X, op=ADD)
                rz_sb = tailp.tile([1, LB], F32, tag="rz")
                nc.vector.reciprocal(out=rz_sb[:], in_=z_sb[:])
                rz64_sb = tailp.tile([1, LB * NK], F32, tag="rz64")
                rza = rz_sb[:]
                rz_b = bass.AP(tensor=rza.tensor, offset=rza.offset,
                               ap=[list(rza.ap[0]), [0, NK], list(rza.ap[1])])
                nc.vector.tensor_copy(
                    out=rz64_sb.rearrange("p (k b) -> p k b", b=LB), in_=rz_b)
                rb_ps = sps.tile([128, LB * NK], F32, tag="tailps")
                nc.tensor.matmul(rb_ps[:], ones[0:1, :], rz64_sb[:],
                                 start=True, stop=True)
                nan_sb = tailp.tile([128, LB * NK], F32, tag="nan")
                nc.vector.tensor_mul(nan_sb[:], na_sb[:], rb_ps[:])

                # transpose to [(k b), t'] and store
                ot_ps = sps.tile([LB * NK, 128], F32, tag="tailps")
                nc.tensor.transpose(ot_ps[:], nan_sb[:], eye32)
                ot_sb = tailp.tile([LB * NK, 128], F32, tag="otsb")
                nc.vector.tensor_copy(out=ot_sb[:], in_=ot_ps[:])
                nc.sync.dma_start(out=out_d[:], in_=ot_sb[:])

            if hw_loop and repeats > 1:
                assert repeats % unroll == 0
                with tc.For_i(0, repeats // unroll, 1):
                    for _u in range(unroll):
                        body()
            else:
                for _rep in range(repeats):
                    body()

    _split_sync_waits(nc)
    return nc


def prep_inputs(inputs: dict) -> list[dict]:
    """Full inputs -> per-core in_maps (host layout prep only)."""
    import ml_dtypes

    enc = np.asarray(inputs["encoder_seq_proj"], np.float32)
    query = np.asarray(inputs["query"], np.float32)
    cum = np.asarray(inputs["cumulative"], np.float32)
    att = np.asarray(inputs["attention"], np.float32)
    alpha = np.asarray(inputs["alpha"], np.float32)
    conv_w = np.asarray(inputs["conv_w"], np.float32)
    L_w = np.asarray(inputs["L_w"], np.float32)
    L_b = np.asarray(inputs["L_b"], np.float32)
    W_w = np.asarray(inputs["W_w"], np.float32)
    W_b = np.asarray(inputs["W_b"], np.float32)
    v_w = np.asarray(inputs["v_w"], np.float32)
    phone_len = np.asarray(inputs["phone_len"], np.int64)

    # folded conv+projection weight: M[c*31+k, a] = sum_f conv_w[f,c,k]*L_w[a,f]
    mcomb = np.einsum("fck,af->cka", conv_w, L_w).reshape(KC, A)
    mcomb = np.ascontiguousarray(mcomb).astype(ml_dtypes.bfloat16)

    # processed query folded into enc (host weight algebra; tiny)
    pq = query @ W_w.T + (W_b + L_b)            # [B, A]
    encq = (enc + pq[:, None, :]).astype(ml_dtypes.bfloat16)

    eye32 = np.eye(128, dtype=np.float32)
    eyeb = np.eye(128, dtype=np.float32).astype(ml_dtypes.bfloat16)
    ones = np.ones((128, 128), np.float32)
    # tri[s,t'] = 1 for t'-2 <= s <= t'  (alpha + shift1 + shift2, in-block)
    idx = np.arange(128)
    dif = idx[None, :] - idx[:, None]          # t' - s
    tri = ((dif >= 0) & (dif <= 2)).astype(np.float32)
    # cor[s,t']: cross-block corner terms from the previous 128-block
    cor = np.zeros((128, 128), np.float32)
    cor[126, 0] = 1.0
    cor[127, 0] = 1.0
    cor[127, 1] = 1.0

    mask = (np.arange(T)[None, :] < phone_len[:, None]).astype(np.float32)

    def lay(arr):  # [8,1024] -> [128, 64] with col = k*8 + b
        return np.ascontiguousarray(
            arr.reshape(LB, NK, 128).transpose(2, 1, 0).reshape(128, LB * NK))

    in_maps = []
    for c in range(NCORES):
        sl = slice(c * LB, (c + 1) * LB)
        cum_c, att_c = cum[sl], att[sl]
        ls = np.zeros((KC, LB, T), ml_dtypes.bfloat16)
        padc = np.zeros((LB, T + 2 * PAD), np.float32)
        pada = np.zeros((LB, T + 2 * PAD), np.float32)
        padc[:, PAD:PAD + T] = cum_c
        pada[:, PAD:PAD + T] = att_c
        for k in range(KW):
            ls[k, :, :] = padc[:, k:k + T]
            ls[KW + k, :, :] = pada[:, k:k + T]

        constblob = np.zeros((128, C_W), np.float32)
        constblob[:, C_TRI:C_TRI + 128] = tri
        constblob[:, C_COR:C_COR + 128] = cor
        constblob[:, C_ONES:C_ONES + 128] = ones
        constblob[:, C_EYE:C_EYE + 128] = eye32
        constblob[:, C_MASK:C_MASK + LB * NK] = lay(mask[sl])
        constblob[:, C_ALPHA:C_ALPHA + LB * NK] = lay(alpha[sl])

        in_maps.append({
            "enc": np.ascontiguousarray(encq[sl]),
            "ls": ls,
            "mcomb": mcomb,
            "vw": np.ascontiguousarray(v_w[0].astype(ml_dtypes.bfloat16)),
            "eyeb": eyeb,
            "constblob": constblob,
        })
    return in_maps


def assemble_output(results: list[dict]) -> np.ndarray:
    out = np.empty((B, 1, T), np.float32)
    for c in range(NCORES):
        oc = results[c]["out"]                      # [(k b), 128]
        oc = oc.reshape(NK, LB, 128).transpose(1, 0, 2).reshape(LB, T)
        out[c * LB:(c + 1) * LB, 0, :] = oc
    return out


_CACHED_NC = None


def kernel(**inputs) -> np.ndarray:
    global _CACHED_NC
    if _CACHED_NC is None:
        _CACHED_NC = build_program(repeats=1)
    in_maps = prep_inputs(inputs)
    res = run_bass_kernel_spmd(_CACHED_NC, in_maps, list(range(NCORES)))
    return assemble_output(res.results)


# revision 24
# speedup vs baseline: 1.0506x; 1.0506x over previous
"""Trainium2 Bass kernel for the forward-attention LSA step (nn_LSA_43404939494068).

Contract: kernel(**inputs) takes the FULL inputs from setup_inputs() and
returns the FULL output [64, 1, 1024] float32. Internally shards batch
across 8 NeuronCores (8 batches each), runs one Bass/Tile program SPMD.

Math notes (vs reference):
  u[b,t]   = sum_a v[a] * tanh(pq[b,a] + enc[b,t,a] + ploc[b,t,a])
  ploc     = conv1d([cumulative; attention]) @ L_w.T + L_b; the conv and the
             L-projection fold into ONE matmul: ploc[t,:] = ls[:,t].T @ M,
             M[(c,k),a] = sum_f conv_w[f,c,k] * L_w[a,f] (host-precomputed
             weight algebra), ls = 62 shifted copies of the two loc rows.
  pq+L_b+W_b is computed on host (tiny: B x A) and folded into enc, which is
  shipped bf16 (halves HBM traffic; final rel-err stays ~1e-3 << 2e-2).
  The reference's division of s=sigmoid(u) by sum(s) cancels exactly in the
  final alpha normalization, so it is skipped.

Engine placement, tuned against wall-differenced hardware timings (the
CoreSim cost model misses the ~400ns weight-load+accumulate cost of a
128-row matmul, which made the enc identity-matmul the real bottleneck):
per [128t x 2x512a] PSUM pair, PE runs the bf16 folded conv+proj matmul;
enc joins either via a bf16 identity-matmul into the same PSUM bank (2 of
4 pairs) or via a DVE tensor_add (other 2 pairs) - the 50/50 split
balances measured PE vs DVE time. One ACT tanh covers each 2-bank pair
(amortizes PSUM access latency); the x*v dot is a DVE
scalar_tensor_tensor with accum writing one u column (Pool cannot run
vector ops on NCv3 - walrus ISA check). The tail (sigmoid, premultiplied
mask*(alpha-band+eps), normalize) runs in a [t',(k,b)] 64-column layout,
then one PE transpose and a single output DMA. Input DMAs are spread
across queues: enc owns SP, first-use constants go on the ACT queue,
the rest on the gpsimd SWDGE queue; the ACT tanh table is preloaded with
a dummy activation at t=0.
"""

import sys

import numpy as np

if "/opt/trn_rl_repo" not in sys.path:
    sys.path.insert(0, "/opt/trn_rl_repo")

import concourse.bass as bass
import concourse.tile as tile
from concourse import mybir
from concourse.bass_utils import run_bass_kernel_spmd

B, T, A = 64, 1024, 512
F, KW = 32, 31
PAD = (KW - 1) // 2
NCORES = 8
LB = B // NCORES          # 8 local batches per core
NK = T // 128             # 8 t-tiles of 128
KC = 62                   # conv contraction = 2 channels * 31 taps
F32 = mybir.dt.float32
F32R = mybir.dt.float32r
BF16 = mybir.dt.bfloat16

# const blob layout (fp32, [128, 640]): tri | cor | ones | eye32 | mask | alpha
C_TRI, C_COR, C_ONES, C_EYE = 0, 128, 256, 384
C_MASK, C_ALPHA = 512, 576
C_W = 640

_MAX_WAITS = 1


def _split_sync_waits(nc):
    """walrus in this toolchain accepts at most one sync-wait per
    instruction; hoist excess waits onto NoOps inserted just before."""
    for fn in nc.m.functions:
        for blk in fn.blocks:
            new_list = []
            for inst in blk.instructions:
                si = inst.sync_info
                if si is not None and si.on_wait and len(si.on_wait) > _MAX_WAITS:
                    waits = list(si.on_wait)
                    extra, keep = waits[:-_MAX_WAITS], waits[-_MAX_WAITS:]
                    for i in range(0, len(extra), _MAX_WAITS):
                        nop = mybir.InstNoOp(
                            name=nc.get_next_instruction_name(),
                            sync_info=mybir.SyncInfo(
                                on_wait=extra[i:i + _MAX_WAITS], on_update=[]
                            ),
                            bass_nofuse=True,
                            engine=inst.engine,
                        )
                        nc.register_instruction(nop)
                        new_list.append(nop)
                    inst.sync_info = mybir.SyncInfo(
                        on_wait=keep, on_update=list(si.on_update)
                    )
                new_list.append(inst)
            blk.instructions[:] = new_list


def build_program(repeats: int = 1, hw_loop: bool = False,
                  stage: str = "full", unroll: int = 1) -> bass.Bass:
    nc = bass.Bass()

    enc_d = nc.declare_dram_parameter("enc", [LB, T, A], BF16, isOutput=False)
    ls_d = nc.declare_dram_parameter("ls", [KC, LB, T], BF16, isOutput=False)
    mcomb_d = nc.declare_dram_parameter("mcomb", [KC, A], BF16, isOutput=False)
    vw_d = nc.declare_dram_parameter("vw", [A], BF16, isOutput=False)
    eyeb_d = nc.declare_dram_parameter("eyeb", [128, 128], BF16, isOutput=False)
    const_d = nc.declare_dram_parameter("constblob", [128, C_W], F32, isOutput=False)
    out_d = nc.declare_dram_parameter("out", [LB * NK, 128], F32, isOutput=True)

    TANH = mybir.ActivationFunctionType.Tanh
    SIG = mybir.ActivationFunctionType.Sigmoid
    IDENT = mybir.ActivationFunctionType.Identity
    MULT = mybir.AluOpType.mult
    ADD = mybir.AluOpType.add

    with tile.TileContext(nc) as tc:
        with (
            tc.tile_pool(name="const", bufs=1) as cpool,
            tc.tile_pool(name="encp", bufs=4) as encp,
            tc.tile_pool(name="xp", bufs=6) as xp,
            tc.tile_pool(name="tailp", bufs=3) as tailp,
            tc.tile_pool(name="zps", bufs=3, space="PSUM") as zps,
            tc.tile_pool(name="sps", bufs=2, space="PSUM") as sps,
        ):
            # ---- constants into SBUF (gpsimd/SWDGE queue; enc owns the SP
            # queue). Order = first-use order: the b0 matmuls need mcomb+eyeb
            # +ls[0] immediately; the const blob is tail-only.
            mcomb_sb = cpool.tile([KC, A], BF16, tag="mcomb")
            nc.scalar.dma_start(out=mcomb_sb[:], in_=mcomb_d[:])
            eyeb_sb = cpool.tile([128, 128], BF16, tag="eyeb")
            nc.scalar.dma_start(out=eyeb_sb[:], in_=eyeb_d[:])

            ls_sb = cpool.tile([KC, LB, T], BF16, tag="ls")
            nc.scalar.dma_start(out=ls_sb[:, 0, :], in_=ls_d[:, 0, :])

            # v broadcast to all 128 partitions (partition-step-0 DMA)
            v_sb = cpool.tile([128, A], BF16, tag="vbc")
            va = vw_d[:]
            v_bcast = bass.AP(tensor=va.tensor, offset=va.offset,
                              ap=[[0, 128]] + [list(p) for p in va.ap])
            nc.scalar.dma_start(out=v_sb[:], in_=v_bcast)

            for b in range(1, LB):
                nc.gpsimd.dma_start(out=ls_sb[:, b, :], in_=ls_d[:, b, :])
            const_sb = cpool.tile([128, C_W], F32, tag="const")
            nc.gpsimd.dma_start(out=const_sb[:], in_=const_d[:])

            u_sb = cpool.tile([128, LB * NK], F32, tag="u")
            eps_sb = cpool.tile([128, 1], F32, tag="eps")
            nc.vector.memset(eps_sb[:], 1e-7)
            warm_sb = cpool.tile([128, 1], F32, tag="warm")
            nc.scalar.activation(out=warm_sb[:], in_=eps_sb[:], func=TANH)

            tri = const_sb[:, C_TRI:C_TRI + 128]
            cor = const_sb[:, C_COR:C_COR + 128]
            ones = const_sb[:, C_ONES:C_ONES + 128]
            eye32 = const_sb[:, C_EYE:C_EYE + 128]
            mask = const_sb[:, C_MASK:C_MASK + LB * NK]
            alpha = const_sb[:, C_ALPHA:C_ALPHA + LB * NK]

            def body():
                # ---- main loop: z = ploc+(pq+enc) ; x = tanh(z) ; u = x.v ----
                for b in range(LB):
                    enc_sb = encp.tile([128, NK, A], BF16, tag="enc")
                    src_enc = enc_d[b].rearrange("(k p) a -> p k a", p=128)
                    if b == 0:
                        nc.sync.dma_start(out=enc_sb[:, 0:2, :],
                                          in_=src_enc[:, 0:2, :])
                        nc.sync.dma_start(out=enc_sb[:, 2:, :],
                                          in_=src_enc[:, 2:, :])
                    else:
                        nc.sync.dma_start(out=enc_sb[:], in_=src_enc)
                    if stage == "dma":
                        continue
                    for kp in range(NK // 2):
                        # two k-tiles share one 2-bank PSUM tile so a single
                        # tanh covers both (amortizes ACT access latency)
                        # dve_add pairs: enc joins via a DVE add instead of
                        # the PE identity-matmul (balances PE vs DVE load)
                        dve_add = ((stage in ("full", "d1") and kp == 0)
                                   or (stage == "fullh" and kp % 2 == 1)
                                   or (stage == "d3" and kp >= 1))
                        z_ps = zps.tile([128, 2, A], F32, tag="z")
                        # both mm1 first, then both identity-matmuls: the
                        # scheduler elides the second (identical) eyeb
                        # weight load when the two are adjacent
                        for j in range(2):
                            k = 2 * kp + j
                            nc.tensor.matmul(
                                z_ps[:, j, :],
                                ls_sb[:, b, k * 128:(k + 1) * 128],
                                mcomb_sb[:],
                                start=True, stop=(dve_add or stage == "noeye"),
                                skip_group_check=True)
                        if not (dve_add or stage == "noeye"):
                            for j in range(2):
                                k = 2 * kp + j
                                nc.tensor.matmul(z_ps[:, j, :], eyeb_sb[:],
                                                 enc_sb[:, k, :],
                                                 start=False, stop=True,
                                                 skip_group_check=True)
                        if stage == "mm":
                            continue
                        x_sb = xp.tile([128, 2, A], BF16, tag="x")
                        if dve_add:
                            xin_sb = xp.tile([128, 2, A], BF16, tag="xin")
                            nc.vector.tensor_add(
                                xin_sb[:], z_ps[:],
                                enc_sb[:, 2 * kp:2 * kp + 2, :])
                            nc.scalar.activation(out=x_sb[:], in_=xin_sb[:],
                                                 func=TANH)
                        else:
                            nc.scalar.activation(out=x_sb[:], in_=z_ps[:],
                                                 func=TANH)
                        if stage == "act":
                            continue
                        for j in range(2):
                            # in-place x *= v: no separate xv tile, so the
                            # stt carries no buffer-rotation wait (x is dead
                            # after this op; the x-pool WAR already orders
                            # the next tanh against it)
                            k = 2 * kp + j
                            col = k * LB + b
                            nc.vector.scalar_tensor_tensor(
                                out=x_sb[:, j, :], in0=x_sb[:, j, :],
                                scalar=1.0, in1=v_sb[:], op0=MULT, op1=MULT,
                                accum_out=u_sb[:, col:col + 1])

                if stage != "full":
                    dum_sb = tailp.tile([LB * NK, 128], F32, tag="otsb")
                    nc.vector.tensor_copy(out=dum_sb[:],
                                          in_=const_sb[0:LB * NK, 0:128])
                    nc.sync.dma_start(out=out_d[:], in_=dum_sb[:])
                    return
                # ---- tail in [t', (k,b)] layout ----
                # wm = (alpha + shift1(alpha) + shift2(alpha) + eps) * mask
                # depends only on constants: runs under the main loop, off
                # the post-u critical path (band shifts via tri/cor matmuls)
                w_ps = sps.tile([128, LB * NK], F32, tag="tailps")
                nc.tensor.matmul(w_ps[:], tri, alpha, start=True, stop=False)
                nc.tensor.matmul(w_ps[:, LB:], cor, alpha[:, :-LB],
                                 start=False, stop=True, skip_group_check=True)
                wm_sb = tailp.tile([128, LB * NK], F32, tag="wm")
                nc.vector.scalar_tensor_tensor(
                    out=wm_sb[:], in0=w_ps[:], scalar=1e-7, in1=mask,
                    op0=ADD, op1=MULT)

                s_sb = tailp.tile([128, LB * NK], F32, tag="s")
                nc.scalar.activation(out=s_sb[:], in_=u_sb[:], func=SIG)
                na_sb = tailp.tile([128, LB * NK], F32, tag="na")
                nc.vector.tensor_mul(na_sb[:], s_sb[:], wm_sb[:])

                # per-batch normalizer: colsum then sum over the k-groups
                cs_ps = sps.tile([1, LB * NK], F32, tag="tailps")
                nc.tensor.matmul(cs_ps[:], ones[:, 0:1], na_sb[:],
                                 start=True, stop=True)
                z_sb = tailp.tile([1, LB], F32, tag="zsum")
                nc.vector.tensor_reduce(
                    out=z_sb[:], in_=cs_ps.rearrange("p (k b) -> p b k", b=LB),
                    axis=mybir.AxisListType.X, op=ADD)
                rz_sb = tailp.tile([1, LB], F32, tag="rz")
                nc.vector.reciprocal(out=rz_sb[:], in_=z_sb[:])
                rz64_sb = tailp.tile([1, LB * NK], F32, tag="rz64")
                rza = rz_sb[:]
                rz_b = bass.AP(tensor=rza.tensor, offset=rza.offset,
                               ap=[list(rza.ap[0]), [0, NK], list(rza.ap[1])])
                nc.vector.tensor_copy(
                    out=rz64_sb.rearrange("p (k b) -> p k b", b=LB), in_=rz_b)
                rb_ps = sps.tile([128, LB * NK], F32, tag="tailps")
                nc.tensor.matmul(rb_ps[:], ones[0:1, :], rz64_sb[:],
                                 start=True, stop=True)
                nan_sb = tailp.tile([128, LB * NK], F32, tag="nan")
                nc.vector.tensor_mul(nan_sb[:], na_sb[:], rb_ps[:])

                # transpose to [(k b), t'] and store
                ot_ps = sps.tile([LB * NK, 128], F32, tag="tailps")
                nc.tensor.transpose(ot_ps[:], nan_sb[:], eye32)
                ot_sb = tailp.tile([LB * NK, 128], F32, tag="otsb")
                nc.vector.tensor_copy(out=ot_sb[:], in_=ot_ps[:])
                nc.sync.dma_start(out=out_d[:], in_=ot_sb[:])

            if hw_loop and repeats > 1:
                assert repeats % unroll == 0
                with tc.For_i(0, repeats // unroll, 1):
                    for _u in range(unroll):
                        body()
            else:
                for _rep in range(repeats):
                    body()

    _split_sync_waits(nc)
    return nc


def prep_inputs(inputs: dict) -> list[dict]:
    """Full inputs -> per-core in_maps (host layout prep only)."""
    import ml_dtypes

    enc = np.asarray(inputs["encoder_seq_proj"], np.float32)
    query = np.asarray(inputs["query"], np.float32)
    cum = np.asarray(inputs["cumulative"], np.float32)
    att = np.asarray(inputs["attention"], np.float32)
    alpha = np.asarray(inputs["alpha"], np.float32)
    conv_w = np.asarray(inputs["conv_w"], np.float32)
    L_w = np.asarray(inputs["L_w"], np.float32)
    L_b = np.asarray(inputs["L_b"], np.float32)
    W_w = np.asarray(inputs["W_w"], np.float32)
    W_b = np.asarray(inputs["W_b"], np.float32)
    v_w = np.asarray(inputs["v_w"], np.float32)
    phone_len = np.asarray(inputs["phone_len"], np.int64)

    # folded conv+projection weight: M[c*31+k, a] = sum_f conv_w[f,c,k]*L_w[a,f]
    mcomb = np.einsum("fck,af->cka", conv_w, L_w).reshape(KC, A)
    mcomb = np.ascontiguousarray(mcomb).astype(ml_dtypes.bfloat16)

    # processed query folded into enc (host weight algebra; tiny)
    pq = query @ W_w.T + (W_b + L_b)            # [B, A]
    encq = (enc + pq[:, None, :]).astype(ml_dtypes.bfloat16)

    eye32 = np.eye(128, dtype=np.float32)
    eyeb = np.eye(128, dtype=np.float32).astype(ml_dtypes.bfloat16)
    ones = np.ones((128, 128), np.float32)
    # tri[s,t'] = 1 for t'-2 <= s <= t'  (alpha + shift1 + shift2, in-block)
    idx = np.arange(128)
    dif = idx[None, :] - idx[:, None]          # t' - s
    tri = ((dif >= 0) & (dif <= 2)).astype(np.float32)
    # cor[s,t']: cross-block corner terms from the previous 128-block
    cor = np.zeros((128, 128), np.float32)
    cor[126, 0] = 1.0
    cor[127, 0] = 1.0
    cor[127, 1] = 1.0

    mask = (np.arange(T)[None, :] < phone_len[:, None]).astype(np.float32)

    def lay(arr):  # [8,1024] -> [128, 64] with col = k*8 + b
        return np.ascontiguousarray(
            arr.reshape(LB, NK, 128).transpose(2, 1, 0).reshape(128, LB * NK))

    in_maps = []
    for c in range(NCORES):
        sl = slice(c * LB, (c + 1) * LB)
        cum_c, att_c = cum[sl], att[sl]
        ls = np.zeros((KC, LB, T), ml_dtypes.bfloat16)
        padc = np.zeros((LB, T + 2 * PAD), np.float32)
        pada = np.zeros((LB, T + 2 * PAD), np.float32)
        padc[:, PAD:PAD + T] = cum_c
        pada[:, PAD:PAD + T] = att_c
        for k in range(KW):
            ls[k, :, :] = padc[:, k:k + T]
            ls[KW + k, :, :] = pada[:, k:k + T]

        constblob = np.zeros((128, C_W), np.float32)
        constblob[:, C_TRI:C_TRI + 128] = tri
        constblob[:, C_COR:C_COR + 128] = cor
        constblob[:, C_ONES:C_ONES + 128] = ones
        constblob[:, C_EYE:C_EYE + 128] = eye32
        constblob[:, C_MASK:C_MASK + LB * NK] = lay(mask[sl])
        constblob[:, C_ALPHA:C_ALPHA + LB * NK] = lay(alpha[sl])

        in_maps.append({
            "enc": np.ascontiguousarray(encq[sl]),
            "ls": ls,
            "mcomb": mcomb,
            "vw": np.ascontiguousarray(v_w[0].astype(ml_dtypes.bfloat16)),
            "eyeb": eyeb,
            "constblob": constblob,
        })
    return in_maps


def assemble_output(results: list[dict]) -> np.ndarray:
    out = np.empty((B, 1, T), np.float32)
    for c in range(NCORES):
        oc = results[c]["out"]                      # [(k b), 128]
        oc = oc.reshape(NK, LB, 128).transpose(1, 0, 2).reshape(LB, T)
        out[c * LB:(c + 1) * LB, 0, :] = oc
    return out


_CACHED_NC = None


def kernel(**inputs) -> np.ndarray:
    global _CACHED_NC
    if _CACHED_NC is None:
        _CACHED_NC = build_program(repeats=1)
    in_maps = prep_inputs(inputs)
    res = run_bass_kernel_spmd(_CACHED_NC, in_maps, list(range(NCORES)))
    return assemble_output(res.results)
